# revision 1
# baseline (speedup 1.0000x reference)
"""Trainium2 Bass kernel for the CMDF block (dense_cnn).

Contract: kernel(**inputs) takes the FULL unsharded inputs (B=8, C=128,
H=W=64) and returns the FULL (8, 128, 64, 64) float32 output.

Sharding: data-parallel over batch — core b computes batch element b.
All weights are replicated (host-side prepacked into matmul layouts).

Math per batch element (see reference):
  Xs   = depthwise3x3(X2, static_w)
  ctx  = relu(w2 @ (w1 @ mean_hw([Xs; Y2])))
  cf   = (w3 @ ctx).reshape(C, 9)          # per-channel dynamic filter
  sf   = ws @ [Xs; Y2]                     # (9, H, W) spatial filter
  dyn  = sum_k shift_k(X2) * (cf[:, k] + sf[k])
  out  = wf[:, :C] @ Xs + wf[:, C:] @ dyn

Kernel strategy (channels on partitions, pixels on the free dim):
  - Xs via 9 accumulating PE matmuls with diag(sw[:, k]) weights over a
    zero-padded X held in SBUF. All large matmuls run in fp32r (full-rate
    fp32 mode, 11-bit mantissa); operands are pre-rounded on the host or
    rounded on-chip by their producing ACT/DVE instruction.
  - sf via matmuls with M=105 (ws replicated into 4 row-groups so the
    per-tap partition-broadcast matmuls can be row-tiled).
  - per tap k: broadcast sf[k] to 128 partitions with a 0/1 "selector"
    matmul, then ONE fused DVE op P_k = (sf_bc + cf[:,k]) * shift_k(X),
    then an accumulating matmul out += wfbT.T @ P_k. The sum over taps
    happens inside the final conv's PSUM accumulation.
"""

import numpy as np

import concourse.bass as bass
import concourse.tile as tile
import concourse.mybir as mybir
from concourse.bass_utils import run_bass_kernel_spmd

B, C, H, W, K = 8, 128, 64, 64, 3
HW = H * W            # 4096
PH, PW = H + 2, W + 2  # 66, 66 padded
NST = 4               # super-tiles over rows
ROWS = H // NST       # 16 image rows per super-tile
STN = ROWS * W        # 1024 pixels per super-tile (2 PSUM banks)
NT = K * K            # 9 taps
MREP = 3 * 32 + NT    # 105: ws replicated at partition groups 0,32,64,96

F32 = mybir.dt.float32
F32R = mybir.dt.float32r
ADD = mybir.AluOpType.add
MULT = mybir.AluOpType.mult
AX = mybir.AxisListType
ACT_COPY = mybir.ActivationFunctionType.Copy
ACT_RELU = mybir.ActivationFunctionType.Relu

_CACHE = {}


def round_f32r(a):
    """Round fp32 to fp32r (RNE at mantissa bit 12) — matches the
    walrus cast_fp32_to_fp32r used by the FP32r matmul datapath."""
    u = np.ascontiguousarray(a, dtype=np.float32).view(np.uint32).astype(np.uint64)
    r = ((u + 0x7FF + ((u >> 12) & 1)) & 0xFFFFF000).astype(np.uint32)
    return r.view(np.float32).reshape(np.asarray(a).shape)


BF16 = mybir.dt.bfloat16


def _absorb(nc, dep_elem, ps_elem):
    """Tiny bf16 matmul that reads one element of `dep_elem` and writes a
    junk element of `ps_elem` (later overwritten by a start=True group).
    Purpose: acquire the semaphore wait on dep_elem's producer on a plain
    (non-fused) matmul, so the following fused f32r matmul — which can
    embed only ONE sem wait — doesn't need two."""
    lh = dep_elem.bitcast(BF16)
    nc.tensor.matmul(ps_elem, lh[:, 0:1], lh[:, 0:1], start=True, stop=True)


def _split_multiwaits(nc):
    """walrus codegen in this toolchain accepts only ONE embedded sem wait
    per instruction. Hoist excess waits onto same-engine NoOps placed
    immediately before the instruction (engines execute in order, so the
    blocking behavior is identical)."""
    ctr = 0
    for fn in nc.m.functions:
        for blk in fn.blocks:
            insts = blk.instructions
            out = []
            for inst in insts:
                si = inst.sync_info
                waits = list(si.on_wait) if si is not None and si.on_wait else []
                if len(waits) > 1:
                    for w in waits[:-1]:
                        ctr += 1
                        out.append(mybir.InstNoOp(
                            name=f"I-wsplit-{ctr}",
                            engine=inst.engine,
                            ins=[], outs=[],
                            sync_info=mybir.SyncInfo(
                                on_wait=[w], on_update=[]),
                        ))
                    inst.sync_info = mybir.SyncInfo(
                        on_wait=[waits[-1]],
                        on_update=list(si.on_update) if si.on_update else [],
                    )
                out.append(inst)
            blk.instructions = out


def _build_bass():
    nc = bass.Bass("TRN2", target_bir_lowering=False, debug=False)

    # single input pack: xpad | y2 | dsw | wsa | wsb | wfa | wfb | bct | w1ab | w2t+w3t
    # one DMA -> one producer proc -> every consumer needs at most one wait
    WR_COLS = NT * C + MREP + MREP + C + C + NT * C  # 2770
    PK_COLS = PH * PW + HW + WR_COLS + 2 * 64 + (64 + NT * C)
    pk = nc.dram_tensor("pk", [C, PK_COLS], F32R, kind="ExternalInput").ap()
    ob = nc.dram_tensor("ob", [C, H, W], F32, kind="ExternalOutput").ap()

    with tile.TileContext(nc) as tc:
        with tc.tile_pool(name="singles", bufs=1) as S:
            stg = S.tile([C, PK_COLS], F32R)
            o = 0
            xpad = stg[:, o : o + PH * PW].rearrange(
                "p (h w) -> p h w", w=PW); o += PH * PW
            y2 = stg[:, o : o + HW]; o += HW
            t_dsw = stg[:, o : o + NT * C]; o += NT * C
            t_wsa = stg[:, o : o + MREP]; o += MREP
            t_wsb = stg[:, o : o + MREP]; o += MREP
            t_wfa = stg[:, o : o + C]; o += C
            t_wfb = stg[:, o : o + C]; o += C
            t_bct = stg[:, o : o + NT * C]; o += NT * C
            t_w1a = stg[:, o : o + 64].bitcast(F32); o += 64
            t_w1b = stg[:, o : o + 64].bitcast(F32); o += 64
            t_w2t = stg[0:64, o : o + 64].bitcast(F32); o += 64
            t_w3t = stg[0:64, o : o + NT * C].bitcast(F32); o += NT * C
            assert o == PK_COLS
            xs = S.tile([C, HW], F32R)
            sfs = S.tile([MREP, HW], F32R)

            xs_parts = S.tile([C, NST], F32)
            y2sum = S.tile([C, 1], F32)
            xs_sum = S.tile([C, 1], F32)
            mxs = S.tile([C, 1], F32)
            my2 = S.tile([C, 1], F32)
            ctx1 = S.tile([64, 1], F32)
            ctx2 = S.tile([64, 1], F32)
            cfsb = S.tile([C, NT], F32)

            # split the input load across DMA queues (the wait-splitter
            # pass makes multi-producer fan-in legal)
            A = PH * PW
            Bc = PH * PW + HW
            nc.sync.dma_start(out=stg[:, 0:A], in_=pk[:, 0:A])
            nc.sync.dma_start(out=stg[:, A:Bc], in_=pk[:, A:Bc])
            nc.sync.dma_start(out=stg[:, Bc:], in_=pk[:, Bc:])

            # mean(Y2) ingredient — DVE is idle during phase A
            nc.vector.tensor_reduce(out=y2sum, in_=y2, axis=AX.X, op=ADD)

            # ---------- phase A: Xs (static depthwise) + sf ----------
            with tc.tile_pool(name="psA", bufs=2, space="PSUM") as psA, \
                 tc.tile_pool(name="psSF", bufs=2, space="PSUM") as psSF:
                for t in range(NST):
                    xs_ps = psA.tile([C, 2, 512], F32, tag="xs_ps")
                    for h in range(2):
                        for k in range(NT):
                            dh, dw = divmod(k, 3)
                            r0 = 16 * t + 8 * h + dh
                            rhs = xpad[:, r0 : r0 + 8, dw : dw + W]
                            nc.tensor.matmul(
                                xs_ps[:, h, :],
                                t_dsw[:, k * C : (k + 1) * C],
                                rhs,
                                start=(k == 0),
                                stop=(k == NT - 1),
                            )
                    nc.scalar.activation(
                        out=xs[:, t * STN : (t + 1) * STN],
                        in_=xs_ps,
                        func=ACT_COPY,
                        accum_out=xs_parts[:, t : t + 1],
                    )
                    sf_ps = psSF.tile([MREP, 2, 512], F32, tag="sf_ps")
                    _absorb(nc, xs[0:1, t * STN : t * STN + 1],
                            sf_ps[0:1, 0, 0:1])
                    for h in range(2):
                        c0 = t * STN + h * 512
                        nc.tensor.matmul(
                            sf_ps[:, h, :],
                            t_wsa,
                            xs[:, c0 : c0 + 512],
                            start=True,
                            stop=False,
                        )
                        nc.tensor.matmul(
                            sf_ps[:, h, :],
                            t_wsb,
                            y2[:, c0 : c0 + 512],
                            start=False,
                            stop=True,
                        )
                    nc.scalar.copy(
                        out=sfs[:, t * STN : (t + 1) * STN], in_=sf_ps
                    )

            # ---------- phase B: context branch -> cf ----------
            with tc.tile_pool(name="psCtx", bufs=1, space="PSUM") as psX:
                nc.vector.tensor_reduce(out=xs_sum, in_=xs_parts, axis=AX.X, op=ADD)
                nc.scalar.mul(out=mxs, in_=xs_sum, mul=1.0 / HW)
                nc.scalar.mul(out=my2, in_=y2sum, mul=1.0 / HW)

                ctx1_ps = psX.tile([64, 1], F32, tag="ctx1")
                _absorb(nc, mxs[0:1, 0:1], ctx1_ps[0:1, 0:1])
                nc.tensor.matmul(ctx1_ps, t_w1a, mxs, start=True, stop=False)
                nc.tensor.matmul(ctx1_ps, t_w1b, my2, start=False, stop=True)
                nc.scalar.copy(out=ctx1, in_=ctx1_ps)

                ctx2_ps = psX.tile([64, 1], F32, tag="ctx2")
                nc.tensor.matmul(ctx2_ps, t_w2t, ctx1, start=True, stop=True)
                nc.scalar.activation(out=ctx2, in_=ctx2_ps, func=ACT_RELU)

                cf_ps = psX.tile([C, NT], F32, tag="cf")
                for k in range(NT):
                    nc.tensor.matmul(
                        cf_ps[:, k : k + 1], t_w3t[:, k * C : (k + 1) * C],
                        ctx2, start=True, stop=True,
                    )
                nc.scalar.copy(out=cfsb, in_=cf_ps)

            # ---------- phase C: dynamic filter + fusion conv ----------
            with tc.tile_pool(name="psBC", bufs=2, space="PSUM") as psBC, \
                 tc.tile_pool(name="psOut", bufs=2, space="PSUM") as psO, \
                 tc.tile_pool(name="pP", bufs=3) as pP, \
                 tc.tile_pool(name="pOsb", bufs=2) as pOsb:
                for t in range(NST):
                    out_ps = psO.tile([C, 2, 8, W], F32, tag="out_ps")
                    _absorb(nc, xs[0:1, t * STN : t * STN + 1],
                            out_ps[0:1, 0, 0, 0:1])
                    for h in range(2):
                        c0 = t * STN + h * 512
                        nc.tensor.matmul(
                            out_ps[:, h],
                            t_wfa,
                            xs[:, c0 : c0 + 512],
                            start=True,
                            stop=False,
                        )
                    for k in range(NT):
                        g = k % 2
                        bc_ps = psBC.tile([C, ROWS, W], F32, tag="bc")
                        if k == 0:
                            _absorb(nc, sfs[0:1, t * STN : t * STN + 1],
                                    bc_ps[0:1, 0, 0:1])
                        for h in range(2):
                            c0 = t * STN + h * 512
                            nc.tensor.matmul(
                                bc_ps[:, 8 * h : 8 * h + 8, :],
                                t_bct[32 * g : 32 * g + NT,
                                      k * C : (k + 1) * C],
                                sfs[32 * g : 32 * g + NT, c0 : c0 + 512],
                                start=True,
                                stop=True,
                                tile_position=(32 * g, 0),
                            )
                        dh, dw = divmod(k, 3)
                        p_sb = pP.tile([C, ROWS, W], F32R, tag="p")
                        nc.vector.scalar_tensor_tensor(
                            out=p_sb,
                            in0=bc_ps,
                            scalar=cfsb[:, k : k + 1],
                            in1=xpad[:, 16 * t + dh : 16 * t + dh + ROWS,
                                     dw : dw + W],
                            op0=ADD,
                            op1=MULT,
                        )
                        for h in range(2):
                            nc.tensor.matmul(
                                out_ps[:, h],
                                t_wfb,
                                p_sb[:, 8 * h : 8 * h + 8, :],
                                start=False,
                                stop=(k == NT - 1),
                            )
                    o_sb = pOsb.tile([C, 2, 8, W], F32, tag="osb")
                    nc.scalar.copy(out=o_sb, in_=out_ps)
                    nc.sync.dma_start(
                        out=ob[:, 16 * t : 16 * t + 16, :],
                        in_=o_sb.rearrange("c b r w -> c (b r) w"),
                    )
    _split_multiwaits(nc)
    return nc


def _prep_weights(static_w, w1, w2, w3, ws, wf):
    """Repack the tiny weights into the SBUF layouts the kernel expects."""
    f = np.float32
    sw = np.ascontiguousarray(static_w.reshape(C, NT), dtype=f)

    dsw = np.zeros((C, NT * C), dtype=f)
    for k in range(NT):
        dsw[np.arange(C), k * C + np.arange(C)] = sw[:, k]

    wsa = np.zeros((C, MREP), dtype=f)
    wsb = np.zeros((C, MREP), dtype=f)
    for g in range(4):
        for k in range(NT):
            wsa[:, 32 * g + k] = ws[k, :C]
            wsb[:, 32 * g + k] = ws[k, C:]

    bct = np.zeros((C, NT * C), dtype=f)
    for g in range(4):
        for k in range(NT):
            bct[32 * g + k, k * C : (k + 1) * C] = 1.0

    wfa = np.ascontiguousarray(wf[:, :C].T, dtype=f)
    wfb = np.ascontiguousarray(wf[:, C:].T, dtype=f)
    wr = round_f32r(
        np.concatenate([dsw, wsa, wsb, wfa, wfb, bct], axis=1)
    )
    wfp = np.concatenate(
        [np.ascontiguousarray(w1[:, :C].T, dtype=f),
         np.ascontiguousarray(w1[:, C:].T, dtype=f)], axis=1
    )
    w3t = np.ascontiguousarray(
        w3.reshape(C, NT, 64).transpose(2, 1, 0), dtype=f
    ).reshape(64, NT * C)
    wg64 = np.concatenate(
        [np.ascontiguousarray(w2.T, dtype=f), w3t], axis=1
    )
    wg = np.zeros((C, wg64.shape[1]), dtype=f)
    wg[:64] = wg64
    return np.concatenate([wr, wfp, wg], axis=1)


def make_in_maps(X2, Y2, static_w, w1, w2, w3, ws, wf):
    wpack = _prep_weights(
        np.asarray(static_w), np.asarray(w1), np.asarray(w2),
        np.asarray(w3), np.asarray(ws), np.asarray(wf),
    )
    X2 = np.asarray(X2)
    Y2 = np.asarray(Y2)
    xpad_all = np.zeros((B, C, PH, PW), dtype=np.float32)
    xpad_all[:, :, 1 : H + 1, 1 : W + 1] = X2
    xpad_all = round_f32r(xpad_all).reshape(B, C, PH * PW)
    y2_all = round_f32r(Y2.reshape(B, C, HW))
    in_maps = []
    for b in range(B):
        m = {"pk": np.ascontiguousarray(np.concatenate(
            [xpad_all[b], y2_all[b], wpack], axis=1))}
        in_maps.append(m)
    return in_maps


def get_nc():
    if "nc" not in _CACHE:
        _CACHE["nc"] = _build_bass()
    return _CACHE["nc"]


def kernel(X2, Y2, static_w, w1, w2, w3, ws, wf):
    nc = get_nc()
    in_maps = make_in_maps(
        np.asarray(X2), np.asarray(Y2), static_w, w1, w2, w3, ws, wf
    )
    res = run_bass_kernel_spmd(nc, in_maps, core_ids=list(range(B)))
    out = np.stack([r["ob"] for r in res.results]).astype(np.float32)
    return out



# revision 2
# speedup vs baseline: 1.0769x; 1.0769x over previous
"""Trainium2 Bass kernel for the CMDF block (dense_cnn).

Contract: kernel(**inputs) takes the FULL unsharded inputs (B=8, C=128,
H=W=64) and returns the FULL (8, 128, 64, 64) float32 output.

Sharding: data-parallel over batch — core b computes batch element b.
All weights are replicated (host-side prepacked into matmul layouts).

Math per batch element (see reference):
  Xs   = depthwise3x3(X2, static_w)
  ctx  = relu(w2 @ (w1 @ mean_hw([Xs; Y2])))
  cf   = (w3 @ ctx).reshape(C, 9)          # per-channel dynamic filter
  sf   = ws @ [Xs; Y2]                     # (9, H, W) spatial filter
  dyn  = sum_k shift_k(X2) * (cf[:, k] + sf[k])
  out  = wf[:, :C] @ Xs + wf[:, C:] @ dyn

Design highlights (vs the 109us v1 baseline):
  - bf16 datapath: all big matmuls bf16 (1 cyc/row, same rate as f32r)
    but the input DMA halves and the DVE tensor_tensor multiply runs in
    the 2x_1p perf mode (0.52 ns/elem vs 1.04).
  - Chunked input DMA ordered by first use (phase-A weights, then X/Y2
    tiles, then phase-C/ctx weights) on both HWDGE queues (SP + ACT).
  - PE warmup stream during the DMA fill: the cost model's p-state ramp
    runs post-idle matmuls at 1.2GHz until the engine has been
    continuously busy 3us; the warmup makes all real work run at 2.4GHz.
  - Phase C per-tap pipeline: broadcast matmuls emitted two taps ahead
    of their consuming wfb matmul (PE executes in order — without this
    wfb_k blocks bc_{k+1} and the elementwise engines starve). ACT
    produces flt_k = bc_k + cf[:,k] (Identity activation, per-partition
    bias) in bf16; DVE multiplies with shift_k(X) at 2x. Two taps per
    tile use the fused DVE scalar_tensor_tensor path instead so neither
    ACT nor DVE exceeds the PE's per-tile time.
  - The two bc matmul pairs of tile 0 run inside the phase-B context
    chain's dependency stalls.
  - Output copies split per half-tile across ACT/DVE, two output DMAs
    per tile on both HWDGE queues (shorter critical tail).
"""

import numpy as np
import ml_dtypes

import concourse.bass as bass
import concourse.tile as tile
import concourse.mybir as mybir
from concourse.bass_utils import run_bass_kernel_spmd

B, C, H, W, K = 8, 128, 64, 64, 3
HW = H * W             # 4096
PH, PW = H + 2, W + 2  # 66, 66 padded
NST = 4                # super-tiles over rows
ROWS = H // NST        # 16 image rows per super-tile
STN = ROWS * W         # 1024 pixels per super-tile
NT = K * K             # 9 taps
MREP = 32 + NT         # 41: selector groups at partitions 0 and 32
NWARM = 8              # PE warmup matmuls (tuned via TimelineSim)

BF16 = mybir.dt.bfloat16
F32 = mybir.dt.float32
ADD = mybir.AluOpType.add
MULT = mybir.AluOpType.mult
AX = mybir.AxisListType
ACT_COPY = mybir.ActivationFunctionType.Copy
ACT_IDENT = mybir.ActivationFunctionType.Identity
ACT_RELU = mybir.ActivationFunctionType.Relu

# Taps whose sf-row broadcast arrives by DMA (SP-issued during phase A,
# zero PE cost, zero chain latency): their filter+multiply runs as one
# fused DVE scalar_tensor_tensor from SBUF. They are processed FIRST in
# each tile so their wfb matmuls flow while the PE-bc taps' longer
# bc->ACT->DVE chains fill. The remaining taps broadcast on the PE and
# go ACT flt (+cf bias, ->bf16) then DVE tensor_tensor at 2x.
DMA_TAPS = (2, 5, 8, 0, 1, 3)
POOL_TT_TAPS = (5, 1)  # these DMA-taps' multiply runs on the Pool engine
ACT_SBUF_TAP = 3       # DMA-tap whose add runs on ACT (DVE relief)
# consumption order: pool-produced taps last (their multiply is slow and
# runs far ahead on the otherwise-idle Pool engine)
TAP_ORDER = (2, 8, 0, 3, 4, 6, 7, 5, 1)
PE_TAPS = tuple(k for k in TAP_ORDER if k not in DMA_TAPS)
USE_DMA_BCAST = True
LEAD = 5               # p-production leads wfb consumption by LEAD taps

# ---- input pack layout (bf16 columns of a [C, PK_COLS] dram tensor) ----
# xpad | y2 | dsw wsa wsb (phase A) | wfa wfb bct (phase C) | ctx f32 tail
# ctx tail (bitcast pairs of bf16 cols): w1a [C,64] | w1b [C,64] |
# w2t [64,64] | w3t [64,1152]
W_DSW = NT * C             # 1152
W_BCT = NT * C             # 1152 (partitions 0..40 used)
CTXF32 = 64 + 64 + 64 + NT * C  # 1344 fp32 columns
WA_COLS = W_DSW + MREP + MREP           # phase A weights chunk
WC_COLS = C + C + W_BCT                 # phase C weights chunk
PK_COLS = PH * PW + HW + WA_COLS + WC_COLS + 2 * CTXF32

_CACHE = {}
DBG = {}


def _lbl(inst, label):
    try:
        DBG[inst.ins.name] = label
    except Exception:
        pass


def _absorb(nc, dep_elem, ps_elem):
    """Tiny bf16 matmul that reads one element of `dep_elem` and writes a
    junk element of `ps_elem` (later overwritten by a start=True group).
    Purpose: acquire the semaphore wait on dep_elem's producer on a plain
    (non-fused) matmul, so the following fused matmul — which can embed
    only ONE sem wait — doesn't need two."""
    lh = dep_elem.bitcast(BF16)
    nc.tensor.matmul(ps_elem, lh[:, 0:1], lh[:, 0:1], start=True, stop=True)


def _split_multiwaits(nc):
    """walrus codegen in this toolchain accepts only ONE embedded sem wait
    per instruction. Hoist excess waits onto same-engine NoOps placed
    immediately before the instruction (engines execute in order, so the
    blocking behavior is identical)."""
    ctr = 0
    for fn in nc.m.functions:
        for blk in fn.blocks:
            insts = blk.instructions
            out = []
            for inst in insts:
                si = inst.sync_info
                waits = list(si.on_wait) if si is not None and si.on_wait else []
                if len(waits) > 1:
                    for w in waits[:-1]:
                        ctr += 1
                        out.append(mybir.InstNoOp(
                            name=f"I-wsplit-{ctr}",
                            engine=inst.engine,
                            ins=[], outs=[],
                            sync_info=mybir.SyncInfo(
                                on_wait=[w], on_update=[]),
                        ))
                    inst.sync_info = mybir.SyncInfo(
                        on_wait=[waits[-1]],
                        on_update=list(si.on_update) if si.on_update else [],
                    )
                out.append(inst)
            blk.instructions = out


def _build_bass():
    nc = bass.Bass("TRN2", target_bir_lowering=False, debug=False)

    pk = nc.dram_tensor("pk", [C, PK_COLS], BF16, kind="ExternalInput").ap()
    ob = nc.dram_tensor("ob", [C, H, W], BF16, kind="ExternalOutput").ap()

    with tile.TileContext(nc) as tc:
        with tc.tile_pool(name="singles", bufs=1) as S:
            stg = S.tile([C, PK_COLS], BF16)
            o = 0
            xpad = stg[:, o:o + PH * PW].rearrange(
                "p (h w) -> p h w", w=PW); o += PH * PW
            y2 = stg[:, o:o + HW]; o += HW
            t_dsw = stg[:, o:o + W_DSW]; o += W_DSW
            t_wsa = stg[:, o:o + MREP]; o += MREP
            t_wsb = stg[:, o:o + MREP]; o += MREP
            t_wfa = stg[:, o:o + C]; o += C
            t_wfb = stg[:, o:o + C]; o += C
            t_bct = stg[:, o:o + W_BCT]; o += W_BCT
            ctxw = stg[:, o:o + 2 * CTXF32].bitcast(F32); o += 2 * CTXF32
            assert o == PK_COLS
            t_w1a = ctxw[:, 0:64]
            t_w1b = ctxw[:, 64:128]
            t_w2t = ctxw[0:64, 128:192]
            t_w3t = ctxw[0:64, 192:192 + NT * C]

            AEND = PH * PW              # xpad end / y2 start
            BEND = PH * PW + HW         # y2 end / phase-A weights start
            CW0 = BEND + WA_COLS        # phase-C weights start
            CTX0 = CW0 + WC_COLS        # ctx weights start

            xs = S.tile([C, HW], BF16)
            sfs = S.tile([MREP, HW], BF16)
            sbc = S.tile([C, NST, len(DMA_TAPS), STN], BF16)
            wconst = S.tile([C, 512], BF16)

            xs_parts = S.tile([C, NST + 1], F32)
            y2parts = S.tile([C, NST], F32)
            y2sum = S.tile([C, 1], F32)
            xs_sum = S.tile([C, 1], F32)
            ctx1 = S.tile([64, 1], F32)
            ctx2 = S.tile([64, 1], F32)
            cfsb = S.tile([C, NT], F32)

            # ---------- input DMA: chunks ordered by first use ----------
            xrow = [0, 18, 34, 50, 66]   # tile t reads xpad rows [16t,16t+18)
            WSAB = BEND + W_DSW          # wsa/wsb start
            nc.sync.dma_start(out=stg[:, BEND:WSAB], in_=pk[:, BEND:WSAB])
            nc.scalar.dma_start(out=stg[:, 0:10 * PW], in_=pk[:, 0:10 * PW])
            nc.sync.dma_start(out=stg[:, 10 * PW:18 * PW],
                              in_=pk[:, 10 * PW:18 * PW])
            nc.scalar.dma_start(out=stg[:, WSAB:CW0], in_=pk[:, WSAB:CW0])
            nc.sync.dma_start(out=stg[:, AEND:AEND + STN],
                              in_=pk[:, AEND:AEND + STN])
            for t in range(1, NST):
                r0, r1 = xrow[t], xrow[t + 1]
                eng = (nc.scalar, nc.sync)[t % 2]
                eng.dma_start(out=stg[:, r0 * PW:r1 * PW],
                              in_=pk[:, r0 * PW:r1 * PW])
                c0 = AEND + t * STN
                eng2 = (nc.sync, nc.scalar)[t % 2]
                eng2.dma_start(out=stg[:, c0:c0 + STN],
                               in_=pk[:, c0:c0 + STN])
            nc.scalar.dma_start(out=stg[:, CW0:CTX0], in_=pk[:, CW0:CTX0])
            nc.sync.dma_start(out=stg[:, CTX0:], in_=pk[:, CTX0:])

            # ---------- PE warmup (p-state ramp) ----------
            nc.vector.memset(wconst, 0.0)
            with tc.tile_pool(name="psW", bufs=1, space="PSUM") as psW:
                wps = psW.tile([C, 512], F32, tag="w")
                for i in range(NWARM):
                    nc.tensor.matmul(wps, wconst[:, 0:128], wconst,
                                     start=True, stop=True)

            # ---------- phase A: Xs (static depthwise) + sf ----------
            with tc.tile_pool(name="psA", bufs=2, space="PSUM") as psA, \
                 tc.tile_pool(name="psSF", bufs=1, space="PSUM") as psSF:
                for t in range(NST):
                    xs_ps = psA.tile([C, 2, 512], F32, tag="xs_ps")
                    for h in range(2):
                        for k in range(NT):
                            dh, dw = divmod(k, 3)
                            r0 = 16 * t + 8 * h + dh
                            rhs = xpad[:, r0:r0 + 8, dw:dw + W]
                            nc.tensor.matmul(
                                xs_ps[:, h, :],
                                t_dsw[:, k * C:(k + 1) * C],
                                rhs,
                                start=(k == 0),
                                stop=(k == NT - 1),
                            )
                    if t < NST - 1:
                        nc.scalar.activation(
                            out=xs[:, t * STN:(t + 1) * STN],
                            in_=xs_ps,
                            func=ACT_COPY,
                            accum_out=xs_parts[:, t:t + 1],
                        )
                    else:
                        # last tile: split the copy so the sf matmuls (and
                        # the exposed phase-B chain behind them) start half
                        # a copy earlier; h1 goes to DVE in parallel.
                        nc.scalar.activation(
                            out=xs[:, t * STN:t * STN + 512],
                            in_=xs_ps[:, 0],
                            func=ACT_COPY,
                            accum_out=xs_parts[:, t:t + 1],
                        )
                        nc.vector.tensor_scalar(
                            out=xs[:, t * STN + 512:(t + 1) * STN],
                            in0=xs_ps[:, 1], scalar1=0.0, scalar2=1.0,
                            op0=ADD, op1=MULT,
                            accum_out=xs_parts[:, t + 1:t + 2],
                        )
                    # Y2 partial sums for the context mean (DVE idle here)
                    nc.vector.tensor_reduce(
                        out=y2parts[:, t:t + 1],
                        in_=y2[:, t * STN:(t + 1) * STN],
                        axis=AX.X, op=ADD)
                    sf_ps = psSF.tile([MREP, 2, 512], F32, tag="sf_ps")
                    _absorb(nc, xs[0:1, t * STN:t * STN + 1],
                            sf_ps[0:1, 0, 0:1])
                    for h in range(2):
                        c0 = t * STN + h * 512
                        nc.tensor.matmul(
                            sf_ps[:, h, :], t_wsa, xs[:, c0:c0 + 512],
                            start=True, stop=False)
                        nc.tensor.matmul(
                            sf_ps[:, h, :], t_wsb, y2[:, c0:c0 + 512],
                            start=False, stop=True)
                    nc.vector.tensor_copy(
                        out=sfs[:, t * STN:(t + 1) * STN], in_=sf_ps)
                    if USE_DMA_BCAST:
                        # DMA-tap broadcasts: SBUF->SBUF DMA with a
                        # 0-stride middle dim replicates one sf row to all
                        # 128 partitions. All on the idle SP queue + idle
                        # DMA engines; lands well before phase C uses it.
                        for i, k in enumerate(DMA_TAPS):
                            src = sfs[k:k + 1,
                                      t * STN:(t + 1) * STN].rearrange(
                                "p (o n) -> p o n", o=1)
                            nc.sync.dma_start(
                                out=sbc[:, t, i],
                                in_=src.broadcast_to((1, C, STN)))

            # ---------- phases B+C interleaved ----------
            # psBC opens before the ctx pool so tile 0's first bc matmuls
            # can run inside the ctx chain's dependency stalls.
            with tc.tile_pool(name="psBC", bufs=2, space="PSUM") as psBC:
                bc_tiles = {}

                def emit_bc(t, k):
                    g = k % 2
                    bc_ps = psBC.tile([C, ROWS, W], F32, tag="bc")
                    if t == 0 and k == 0:
                        _absorb(nc, sfs[0:1, 0:1], bc_ps[0:1, 0, 0:1])
                    for h in range(2):
                        c0 = t * STN + h * 512
                        _lbl(nc.tensor.matmul(
                            bc_ps[:, 8 * h:8 * h + 8, :],
                            t_bct[32 * g:32 * g + NT, k * C:(k + 1) * C],
                            sfs[32 * g:32 * g + NT, c0:c0 + 512],
                            start=True, stop=True,
                            tile_position=(32 * g, 0),
                        ), f"bc({t},{k},{h})")
                    bc_tiles[(t, k)] = bc_ps



                # ---- phase B: context branch -> cf ----
                with tc.tile_pool(name="psCtx", bufs=1, space="PSUM") as psX:
                    nc.vector.tensor_reduce(out=xs_sum, in_=xs_parts,
                                            axis=AX.X, op=ADD)
                    nc.vector.tensor_reduce(out=y2sum, in_=y2parts,
                                            axis=AX.X, op=ADD)
                    ctxp = psX.tile([C, 16], F32, tag="ctx")
                    ctx1_ps = ctxp[0:64, 0:1]
                    _absorb(nc, xs_sum[0:1, 0:1], ctxp[0:1, 15:16])
                    nc.tensor.matmul(ctx1_ps, t_w1a, xs_sum,
                                     start=True, stop=False)
                    nc.tensor.matmul(ctx1_ps, t_w1b, y2sum,
                                     start=False, stop=True)
                    nc.scalar.copy(out=ctx1, in_=ctx1_ps)

                    ctx2_ps = ctxp[0:64, 1:2]
                    nc.tensor.matmul(ctx2_ps, t_w2t, ctx1,
                                     start=True, stop=True)
                    nc.scalar.activation(out=ctx2, in_=ctx2_ps,
                                         func=ACT_RELU)

                    cf_ps = ctxp[:, 2:2 + NT]
                    for k in range(NT):
                        nc.tensor.matmul(
                            cf_ps[:, k:k + 1],
                            t_w3t[:, k * C:(k + 1) * C],
                            ctx2, start=True, stop=True)
                    nc.scalar.copy(out=cfsb, in_=cf_ps)

                # ---- phase C: dynamic filter + fusion conv ----
                # Software pipeline over the flat (tile, tap) sequence:
                # p-production (bc/DMA-broadcast + filter + multiply) runs
                # LEAD taps ahead of the consuming wfb matmuls so the
                # elementwise engines' chain latency never stalls the PE,
                # including across tile boundaries.
                with tc.tile_pool(name="psOut", bufs=2, space="PSUM") as psO, \
                     tc.tile_pool(name="pF", bufs=LEAD + 1) as pF, \
                     tc.tile_pool(name="pP", bufs=LEAD + 1) as pP, \
                     tc.tile_pool(name="pFq", bufs=4) as pFq, \
                     tc.tile_pool(name="pPq", bufs=4) as pPq, \
                     tc.tile_pool(name="pOsb", bufs=2) as pOsb:
                    # consumption sequence: TAP_ORDER per tile, pool last.
                    # production sequence: pool-taps hoisted one tile early
                    # (their inputs are phase-A products, and the Pool's
                    # slow multiply needs the head start).
                    # tile 3's drain: consume the slow-produced ACT-flt
                    # taps mid-tile and the fast DVE-ts taps last, so the
                    # pipeline wind-down stays PE-bound.
                    TAP_ORDER_L = (2, 8, 4, 6, 7, 0, 3, 5, 1)
                    def order_of(t):
                        return TAP_ORDER_L if t == NST - 1 else TAP_ORDER
                    cons_seq = [(t, k) for t in range(NST)
                                for k in order_of(t)]
                    prod_seq = []
                    for t in range(NST):
                        reg = [k for k in order_of(t)
                               if k not in POOL_TT_TAPS]
                        # first three regular taps lead (shortest chain
                        # to the first wfbs), then the pool taps
                        prod_seq += [(t, k) for k in reg[:3]]
                        if t == 0:
                            prod_seq += [(0, k) for k in POOL_TT_TAPS]
                        prod_seq += [(t, k) for k in reg[3:]]
                        if t + 1 < NST:
                            prod_seq += [(t + 1, k)
                                         for k in POOL_TT_TAPS]
                    p_tiles = {}
                    out_tiles = {}

                    def produce(t, k):
                        dh, dw = divmod(k, 3)
                        xwin = xpad[:, 16 * t + dh:16 * t + dh + ROWS,
                                    dw:dw + W]
                        pool_tap = k in POOL_TT_TAPS
                        pPx = pPq if pool_tap else pP
                        pFx = pFq if pool_tap else pF
                        p_sb = pPx.tile([C, ROWS, W], BF16, tag="p")
                        if USE_DMA_BCAST and k in DMA_TAPS:
                            # broadcast arrived by DMA
                            di = DMA_TAPS.index(k)
                            sb = sbc[:, t, di].rearrange(
                                "c (r w) -> c r w", w=W)
                            flt = pFx.tile([C, ROWS, W], BF16, tag="flt")
                            if k == ACT_SBUF_TAP or (t == 0 and k == 8):
                                # ACT takes this add for DVE/ACT balance
                                _lbl(nc.scalar.activation(
                                    out=flt, in_=sb, func=ACT_IDENT,
                                    bias=cfsb[:, k:k + 1]),
                                     f"fltA({t},{k})")
                            else:
                                _lbl(nc.vector.tensor_scalar(
                                    out=flt, in0=sb,
                                    scalar1=cfsb[:, k:k + 1],
                                    scalar2=1.0, op0=ADD, op1=MULT),
                                     f"ts({t},{k})")
                            eng = nc.gpsimd if pool_tap else nc.vector
                            _lbl(eng.tensor_tensor(
                                out=p_sb, in0=flt, in1=xwin, op=MULT),
                                 f"tt({t},{k})")
                        else:
                            emit_bc(t, k)
                            bc_ps = bc_tiles.pop((t, k))
                            if k in DMA_TAPS:
                                _lbl(nc.vector.scalar_tensor_tensor(
                                    out=p_sb, in0=bc_ps,
                                    scalar=cfsb[:, k:k + 1], in1=xwin,
                                    op0=ADD, op1=MULT), f"stt({t},{k})")
                            else:
                                # ACT adds cf (bias) -> bf16 SBUF, DVE
                                # multiplies at 2x
                                flt = pFx.tile([C, ROWS, W], BF16,
                                               tag="flt")
                                _lbl(nc.scalar.activation(
                                    out=flt, in_=bc_ps, func=ACT_IDENT,
                                    bias=cfsb[:, k:k + 1]),
                                     f"fltP({t},{k})")
                                _lbl(nc.vector.tensor_tensor(
                                    out=p_sb, in0=flt, in1=xwin,
                                    op=MULT), f"tt({t},{k})")
                        p_tiles[(t, k)] = p_sb

                    def consume(t, k, ki):
                        if ki == 0:
                            ohs = []
                            for h in range(2):
                                out_ps = psO.tile([C, 8, W], F32,
                                                  tag=f"out{h}")
                                ohs.append(out_ps)
                                if h == 0:
                                    _absorb(
                                        nc,
                                        xs[0:1, t * STN:t * STN + 1],
                                        out_ps[0:1, 0, 0:1])
                                c0 = t * STN + h * 512
                                nc.tensor.matmul(
                                    out_ps, t_wfa, xs[:, c0:c0 + 512],
                                    start=True, stop=False)
                            out_tiles[t] = ohs
                        ohs = out_tiles[t]
                        p_sb = p_tiles.pop((t, k))
                        for h in range(2):
                            _lbl(nc.tensor.matmul(
                                ohs[h], t_wfb,
                                p_sb[:, 8 * h:8 * h + 8, :],
                                start=False, stop=(ki == NT - 1)),
                                 f"wfb({t},{k},{h})")
                        if ki == NT - 1:
                            last = t == NST - 1
                            for h in range(2):
                                o_sb = pOsb.tile([C, 8, W], BF16,
                                                 tag=f"osb{h}")
                                if last and h == 1:
                                    _lbl(nc.vector.tensor_copy(
                                        out=o_sb, in_=ohs[h]),
                                         f"osb({t},{h})")
                                else:
                                    _lbl(nc.scalar.copy(out=o_sb,
                                                        in_=ohs[h]),
                                         f"osb({t},{h})")
                                # last tile: DMAs on both HWDGE queues so
                                # their issue overheads overlap
                                eng = nc.scalar if (last and h == 1) \
                                    else nc.sync
                                _lbl(eng.dma_start(
                                    out=ob[:, 16 * t + 8 * h:
                                           16 * t + 8 * h + 8, :],
                                    in_=o_sb.rearrange(
                                        "c r w -> c (r w)"),
                                ), f"outdma({t},{h})")

                    for j in range(len(prod_seq) + LEAD):
                        if j < len(prod_seq):
                            produce(*prod_seq[j])
                        if j >= LEAD:
                            t, k = cons_seq[j - LEAD]
                            consume(t, k, (j - LEAD) % NT)
    _split_multiwaits(nc)
    return nc


def _bf16(a):
    return np.asarray(a, dtype=np.float32).astype(ml_dtypes.bfloat16)


def _prep_weights(static_w, w1, w2, w3, ws, wf):
    """Repack the tiny weights into the SBUF layouts the kernel expects.
    Returns the [C, WA_COLS + WC_COLS + 2*CTXF32] bf16 weight tail."""
    f = np.float32
    sw = np.ascontiguousarray(static_w.reshape(C, NT), dtype=f)

    dsw = np.zeros((C, NT * C), dtype=f)
    for k in range(NT):
        dsw[np.arange(C), k * C + np.arange(C)] = sw[:, k]

    wsa = np.zeros((C, MREP), dtype=f)
    wsb = np.zeros((C, MREP), dtype=f)
    for g in range(2):
        for k in range(NT):
            wsa[:, 32 * g + k] = ws[k, :C]
            wsb[:, 32 * g + k] = ws[k, C:]

    wfa = np.ascontiguousarray(wf[:, :C].T, dtype=f)
    wfb = np.ascontiguousarray(wf[:, C:].T, dtype=f)

    bct = np.zeros((C, NT * C), dtype=f)
    for g in range(2):
        for k in range(NT):
            bct[32 * g + k, k * C:(k + 1) * C] = 1.0

    tail16 = _bf16(np.concatenate([dsw, wsa, wsb, wfa, wfb, bct], axis=1))

    # fp32 context weights (1/HW folded into w1), bitcast into bf16 pairs
    w1s = np.asarray(w1, dtype=f) * (1.0 / HW)
    ctx = np.zeros((C, CTXF32), dtype=f)
    ctx[:, 0:64] = w1s[:, :C].T
    ctx[:, 64:128] = w1s[:, C:].T
    ctx[0:64, 128:192] = np.asarray(w2, dtype=f).T
    w3t = np.ascontiguousarray(
        np.asarray(w3, dtype=f).reshape(C, NT, 64).transpose(2, 1, 0)
    ).reshape(64, NT * C)
    ctx[0:64, 192:192 + NT * C] = w3t
    ctx16 = np.ascontiguousarray(ctx).view(ml_dtypes.bfloat16)

    return np.concatenate([tail16, ctx16], axis=1)


def make_in_maps(X2, Y2, static_w, w1, w2, w3, ws, wf):
    wpack = _prep_weights(
        np.asarray(static_w), np.asarray(w1), np.asarray(w2),
        np.asarray(w3), np.asarray(ws), np.asarray(wf),
    )
    X2 = np.asarray(X2)
    Y2 = np.asarray(Y2)
    xpad_all = np.zeros((B, C, PH, PW), dtype=np.float32)
    xpad_all[:, :, 1:H + 1, 1:W + 1] = X2
    xpad_all = _bf16(xpad_all).reshape(B, C, PH * PW)
    y2_all = _bf16(Y2.reshape(B, C, HW))
    in_maps = []
    for b in range(B):
        m = {"pk": np.ascontiguousarray(np.concatenate(
            [xpad_all[b], y2_all[b], wpack], axis=1))}
        in_maps.append(m)
    return in_maps


def get_nc():
    if "nc" not in _CACHE:
        _CACHE["nc"] = _build_bass()
    return _CACHE["nc"]


def kernel(X2, Y2, static_w, w1, w2, w3, ws, wf):
    nc = get_nc()
    in_maps = make_in_maps(
        np.asarray(X2), np.asarray(Y2), static_w, w1, w2, w3, ws, wf)
    res = run_bass_kernel_spmd(nc, in_maps, core_ids=list(range(B)))
    out = np.stack([np.asarray(r["ob"]).astype(np.float32)
                    for r in res.results])
    return out


# revision 3
# speedup vs baseline: 1.1060x; 1.0271x over previous
"""Trainium2 Bass kernel for the CMDF block (dense_cnn).

Contract: kernel(**inputs) takes the FULL unsharded inputs (B=8, C=128,
H=W=64) and returns the FULL (8, 128, 64, 64) float32 output.

Sharding: data-parallel over batch — core b computes batch element b.
All weights are replicated (host-side prepacked into matmul layouts).

Math per batch element (see reference):
  Xs   = depthwise3x3(X2, static_w)
  ctx  = relu(w2 @ (w1 @ mean_hw([Xs; Y2])))
  cf   = (w3 @ ctx).reshape(C, 9)          # per-channel dynamic filter
  sf   = ws @ [Xs; Y2]                     # (9, H, W) spatial filter
  dyn  = sum_k shift_k(X2) * (cf[:, k] + sf[k])
  out  = wf[:, :C] @ Xs + wf[:, C:] @ dyn

Design highlights (vs the 109us v1 baseline):
  - bf16 datapath: all big matmuls bf16 (1 cyc/row, same rate as f32r)
    but the input DMA halves and the DVE tensor_tensor multiply runs in
    the 2x_1p perf mode (0.52 ns/elem vs 1.04).
  - Chunked input DMA ordered by first use (phase-A weights, then X/Y2
    tiles, then phase-C/ctx weights) on both HWDGE queues (SP + ACT).
  - PE warmup stream during the DMA fill: the cost model's p-state ramp
    runs post-idle matmuls at 1.2GHz until the engine has been
    continuously busy 3us; the warmup makes all real work run at 2.4GHz.
  - Phase C per-tap pipeline: broadcast matmuls emitted two taps ahead
    of their consuming wfb matmul (PE executes in order — without this
    wfb_k blocks bc_{k+1} and the elementwise engines starve). ACT
    produces flt_k = bc_k + cf[:,k] (Identity activation, per-partition
    bias) in bf16; DVE multiplies with shift_k(X) at 2x. Two taps per
    tile use the fused DVE scalar_tensor_tensor path instead so neither
    ACT nor DVE exceeds the PE's per-tile time.
  - The two bc matmul pairs of tile 0 run inside the phase-B context
    chain's dependency stalls.
  - Output copies split per half-tile across ACT/DVE, two output DMAs
    per tile on both HWDGE queues (shorter critical tail).
"""

import numpy as np
import ml_dtypes

import concourse.bass as bass
import concourse.tile as tile
import concourse.mybir as mybir
from concourse.bass_utils import run_bass_kernel_spmd

B, C, H, W, K = 8, 128, 64, 64, 3
HW = H * W             # 4096
PH, PW = H + 2, W + 2  # 66, 66 padded
NST = 4                # super-tiles over rows
ROWS = H // NST        # 16 image rows per super-tile
STN = ROWS * W         # 1024 pixels per super-tile
NT = K * K             # 9 taps
MREP = 32 + NT         # 41: selector groups at partitions 0 and 32
NWARM = 6              # PE warmup matmuls (tuned via TimelineSim)

BF16 = mybir.dt.bfloat16
F32 = mybir.dt.float32
ADD = mybir.AluOpType.add
MULT = mybir.AluOpType.mult
AX = mybir.AxisListType
ACT_COPY = mybir.ActivationFunctionType.Copy
ACT_IDENT = mybir.ActivationFunctionType.Identity
ACT_RELU = mybir.ActivationFunctionType.Relu

# Taps whose sf-row broadcast arrives by DMA (SP-issued during phase A,
# zero PE cost, zero chain latency): their filter+multiply runs as one
# fused DVE scalar_tensor_tensor from SBUF. They are processed FIRST in
# each tile so their wfb matmuls flow while the PE-bc taps' longer
# bc->ACT->DVE chains fill. The remaining taps broadcast on the PE and
# go ACT flt (+cf bias, ->bf16) then DVE tensor_tensor at 2x.
DMA_TAPS = (2, 5, 8, 0, 1, 3)
POOL_TT_TAPS = (5, 1)  # these DMA-taps' multiply runs on the Pool engine
ACT_SBUF_TAP = 3       # DMA-tap whose add runs on ACT (DVE relief)
# consumption order: pool-produced taps last (their multiply is slow and
# runs far ahead on the otherwise-idle Pool engine)
TAP_ORDER = (2, 8, 0, 3, 4, 6, 7, 5, 1)
PE_TAPS = tuple(k for k in TAP_ORDER if k not in DMA_TAPS)
USE_DMA_BCAST = True
LEAD = 9               # p-production leads wfb consumption by LEAD taps

# ---- input pack layout (bf16 columns of a [C, PK_COLS] dram tensor) ----
# xpad | y2 | dsw wsa wsb (phase A) | wfa wfb bct (phase C) | ctx f32 tail
# ctx tail (bitcast pairs of bf16 cols): w1a [C,64] | w1b [C,64] |
# w2t [64,64] | w3t [64,1152]
W_DSW = NT * C             # 1152
W_BCT = NT * C             # 1152 (partitions 0..40 used)
CTXF32 = 64 + 64 + 64 + NT * C  # 1344 fp32 columns
WA_COLS = W_DSW + MREP + MREP           # phase A weights chunk
WC_COLS = C + C + W_BCT                 # phase C weights chunk
PK_COLS = PH * PW + HW + WA_COLS + WC_COLS + 2 * CTXF32

_CACHE = {}
DBG = {}


def _lbl(inst, label):
    try:
        DBG[inst.ins.name] = label
    except Exception:
        pass


def _absorb(nc, dep_elem, ps_elem):
    """Tiny bf16 matmul that reads one element of `dep_elem` and writes a
    junk element of `ps_elem` (later overwritten by a start=True group).
    Purpose: acquire the semaphore wait on dep_elem's producer on a plain
    (non-fused) matmul, so the following fused matmul — which can embed
    only ONE sem wait — doesn't need two."""
    lh = dep_elem.bitcast(BF16)
    nc.tensor.matmul(ps_elem, lh[:, 0:1], lh[:, 0:1], start=True, stop=True)


def _split_multiwaits(nc):
    """walrus codegen in this toolchain accepts only ONE embedded sem wait
    per instruction. Hoist excess waits onto same-engine NoOps placed
    immediately before the instruction (engines execute in order, so the
    blocking behavior is identical)."""
    ctr = 0
    for fn in nc.m.functions:
        for blk in fn.blocks:
            insts = blk.instructions
            out = []
            for inst in insts:
                si = inst.sync_info
                waits = list(si.on_wait) if si is not None and si.on_wait else []
                if len(waits) > 1:
                    for w in waits[:-1]:
                        ctr += 1
                        out.append(mybir.InstNoOp(
                            name=f"I-wsplit-{ctr}",
                            engine=inst.engine,
                            ins=[], outs=[],
                            sync_info=mybir.SyncInfo(
                                on_wait=[w], on_update=[]),
                        ))
                    inst.sync_info = mybir.SyncInfo(
                        on_wait=[waits[-1]],
                        on_update=list(si.on_update) if si.on_update else [],
                    )
                out.append(inst)
            blk.instructions = out


def _build_bass():
    nc = bass.Bass("TRN2", target_bir_lowering=False, debug=False)

    pk = nc.dram_tensor("pk", [C, PK_COLS], BF16, kind="ExternalInput").ap()
    ob = nc.dram_tensor("ob", [C, H, W], BF16, kind="ExternalOutput").ap()

    with tile.TileContext(nc) as tc:
        with tc.tile_pool(name="singles", bufs=1) as S:
            stg = S.tile([C, PK_COLS], BF16)
            o = 0
            xpad = stg[:, o:o + PH * PW].rearrange(
                "p (h w) -> p h w", w=PW); o += PH * PW
            y2 = stg[:, o:o + HW]; o += HW
            t_dsw = stg[:, o:o + W_DSW]; o += W_DSW
            t_wsa = stg[:, o:o + MREP]; o += MREP
            t_wsb = stg[:, o:o + MREP]; o += MREP
            t_wfa = stg[:, o:o + C]; o += C
            t_wfb = stg[:, o:o + C]; o += C
            t_bct = stg[:, o:o + W_BCT]; o += W_BCT
            ctxw = stg[:, o:o + 2 * CTXF32].bitcast(F32); o += 2 * CTXF32
            assert o == PK_COLS
            t_w1a = ctxw[:, 0:64]
            t_w1b = ctxw[:, 64:128]
            t_w2t = ctxw[0:64, 128:192]
            t_w3t = ctxw[0:64, 192:192 + NT * C]

            AEND = PH * PW              # xpad end / y2 start
            BEND = PH * PW + HW         # y2 end / phase-A weights start
            CW0 = BEND + WA_COLS        # phase-C weights start
            CTX0 = CW0 + WC_COLS        # ctx weights start

            xs = S.tile([C, HW], BF16)
            sfs = S.tile([MREP, HW], BF16)
            sbc = S.tile([C, NST, len(DMA_TAPS), STN], BF16)
            wconst = S.tile([C, 512], BF16)

            xs_parts = S.tile([C, NST + 1], F32)
            y2parts = S.tile([C, NST], F32)
            y2sum = S.tile([C, 1], F32)
            xs_sum = S.tile([C, 1], F32)
            ctx1 = S.tile([64, 1], F32)
            ctx2 = S.tile([64, 1], F32)
            cfsb = S.tile([C, NT], F32)

            # ---------- input DMA: chunks ordered by first use ----------
            xrow = [0, 18, 34, 50, 66]   # tile t reads xpad rows [16t,16t+18)
            WSAB = BEND + W_DSW          # wsa/wsb start
            nc.sync.dma_start(out=stg[:, BEND:WSAB], in_=pk[:, BEND:WSAB])
            nc.scalar.dma_start(out=stg[:, 0:10 * PW], in_=pk[:, 0:10 * PW])
            nc.sync.dma_start(out=stg[:, 10 * PW:18 * PW],
                              in_=pk[:, 10 * PW:18 * PW])
            nc.scalar.dma_start(out=stg[:, WSAB:CW0], in_=pk[:, WSAB:CW0])
            nc.sync.dma_start(out=stg[:, AEND:AEND + STN],
                              in_=pk[:, AEND:AEND + STN])
            for t in range(1, NST):
                r0, r1 = xrow[t], xrow[t + 1]
                eng = (nc.scalar, nc.sync)[t % 2]
                eng.dma_start(out=stg[:, r0 * PW:r1 * PW],
                              in_=pk[:, r0 * PW:r1 * PW])
                c0 = AEND + t * STN
                eng2 = (nc.sync, nc.scalar)[t % 2]
                eng2.dma_start(out=stg[:, c0:c0 + STN],
                               in_=pk[:, c0:c0 + STN])
            nc.scalar.dma_start(out=stg[:, CW0:CTX0], in_=pk[:, CW0:CTX0])
            nc.sync.dma_start(out=stg[:, CTX0:], in_=pk[:, CTX0:])

            # ---------- PE warmup (p-state ramp) ----------
            nc.vector.memset(wconst, 0.0)
            with tc.tile_pool(name="psW", bufs=1, space="PSUM") as psW:
                wps = psW.tile([C, 512], F32, tag="w")
                for i in range(NWARM):
                    nc.tensor.matmul(wps, wconst[:, 0:128], wconst,
                                     start=True, stop=True)

            # ---------- phase A: Xs (static depthwise) + sf ----------
            with tc.tile_pool(name="psA", bufs=2, space="PSUM") as psA, \
                 tc.tile_pool(name="psSF", bufs=1, space="PSUM") as psSF:
                for t in range(NST):
                    xs_ps = psA.tile([C, 2, 512], F32, tag="xs_ps")
                    for h in range(2):
                        for k in range(NT):
                            dh, dw = divmod(k, 3)
                            r0 = 16 * t + 8 * h + dh
                            rhs = xpad[:, r0:r0 + 8, dw:dw + W]
                            nc.tensor.matmul(
                                xs_ps[:, h, :],
                                t_dsw[:, k * C:(k + 1) * C],
                                rhs,
                                start=(k == 0),
                                stop=(k == NT - 1),
                            )
                    if t < NST - 1:
                        nc.scalar.activation(
                            out=xs[:, t * STN:(t + 1) * STN],
                            in_=xs_ps,
                            func=ACT_COPY,
                            accum_out=xs_parts[:, t:t + 1],
                        )
                    else:
                        # last tile: split the copy so the sf matmuls (and
                        # the exposed phase-B chain behind them) start half
                        # a copy earlier; h1 goes to DVE in parallel.
                        nc.scalar.activation(
                            out=xs[:, t * STN:t * STN + 512],
                            in_=xs_ps[:, 0],
                            func=ACT_COPY,
                            accum_out=xs_parts[:, t:t + 1],
                        )
                        nc.vector.tensor_scalar(
                            out=xs[:, t * STN + 512:(t + 1) * STN],
                            in0=xs_ps[:, 1], scalar1=0.0, scalar2=1.0,
                            op0=ADD, op1=MULT,
                            accum_out=xs_parts[:, t + 1:t + 2],
                        )
                    # Y2 partial sums for the context mean (DVE idle here)
                    nc.vector.tensor_reduce(
                        out=y2parts[:, t:t + 1],
                        in_=y2[:, t * STN:(t + 1) * STN],
                        axis=AX.X, op=ADD)
                    sf_ps = psSF.tile([MREP, 2, 512], F32, tag="sf_ps")
                    _absorb(nc, xs[0:1, t * STN:t * STN + 1],
                            sf_ps[0:1, 0, 0:1])
                    for h in range(2):
                        c0 = t * STN + h * 512
                        nc.tensor.matmul(
                            sf_ps[:, h, :], t_wsa, xs[:, c0:c0 + 512],
                            start=True, stop=False)
                        nc.tensor.matmul(
                            sf_ps[:, h, :], t_wsb, y2[:, c0:c0 + 512],
                            start=False, stop=True)
                    nc.vector.tensor_copy(
                        out=sfs[:, t * STN:(t + 1) * STN], in_=sf_ps)
                    if USE_DMA_BCAST:
                        # DMA-tap broadcasts: SBUF->SBUF DMA with a
                        # 0-stride middle dim replicates one sf row to all
                        # 128 partitions. All on the idle SP queue + idle
                        # DMA engines; lands well before phase C uses it.
                        for i, k in enumerate(DMA_TAPS):
                            src = sfs[k:k + 1,
                                      t * STN:(t + 1) * STN].rearrange(
                                "p (o n) -> p o n", o=1)
                            nc.sync.dma_start(
                                out=sbc[:, t, i],
                                in_=src.broadcast_to((1, C, STN)))

            # ---------- phases B+C interleaved ----------
            # psBC opens before the ctx pool so tile 0's first bc matmuls
            # can run inside the ctx chain's dependency stalls.
            with tc.tile_pool(name="psBC", bufs=2, space="PSUM") as psBC:
                bc_tiles = {}

                def emit_bc(t, k):
                    g = k % 2
                    bc_ps = psBC.tile([C, ROWS, W], F32, tag="bc")
                    if t == 0 and k == 0:
                        _absorb(nc, sfs[0:1, 0:1], bc_ps[0:1, 0, 0:1])
                    for h in range(2):
                        c0 = t * STN + h * 512
                        _lbl(nc.tensor.matmul(
                            bc_ps[:, 8 * h:8 * h + 8, :],
                            t_bct[32 * g:32 * g + NT, k * C:(k + 1) * C],
                            sfs[32 * g:32 * g + NT, c0:c0 + 512],
                            start=True, stop=True,
                            tile_position=(32 * g, 0),
                        ), f"bc({t},{k},{h})")
                    bc_tiles[(t, k)] = bc_ps



                # ---- phase B: context branch -> cf ----
                with tc.tile_pool(name="psCtx", bufs=1, space="PSUM") as psX:
                    nc.vector.tensor_reduce(out=xs_sum, in_=xs_parts,
                                            axis=AX.X, op=ADD)
                    nc.vector.tensor_reduce(out=y2sum, in_=y2parts,
                                            axis=AX.X, op=ADD)
                    ctxp = psX.tile([C, 16], F32, tag="ctx")
                    ctx1_ps = ctxp[0:64, 0:1]
                    _absorb(nc, xs_sum[0:1, 0:1], ctxp[0:1, 15:16])
                    nc.tensor.matmul(ctx1_ps, t_w1a, xs_sum,
                                     start=True, stop=False)
                    nc.tensor.matmul(ctx1_ps, t_w1b, y2sum,
                                     start=False, stop=True)
                    nc.scalar.copy(out=ctx1, in_=ctx1_ps)

                    ctx2_ps = ctxp[0:64, 1:2]
                    nc.tensor.matmul(ctx2_ps, t_w2t, ctx1,
                                     start=True, stop=True)
                    nc.scalar.activation(out=ctx2, in_=ctx2_ps,
                                         func=ACT_RELU)

                    cf_ps = ctxp[:, 2:2 + NT]
                    for k in range(NT):
                        nc.tensor.matmul(
                            cf_ps[:, k:k + 1],
                            t_w3t[:, k * C:(k + 1) * C],
                            ctx2, start=True, stop=True)
                    nc.scalar.copy(out=cfsb, in_=cf_ps)

                # ---- phase C: dynamic filter + fusion conv ----
                # Software pipeline over the flat (tile, tap) sequence:
                # p-production (bc/DMA-broadcast + filter + multiply) runs
                # LEAD taps ahead of the consuming wfb matmuls so the
                # elementwise engines' chain latency never stalls the PE,
                # including across tile boundaries.
                with tc.tile_pool(name="psOut", bufs=2, space="PSUM") as psO, \
                     tc.tile_pool(name="pF", bufs=LEAD + 1) as pF, \
                     tc.tile_pool(name="pP", bufs=LEAD + 1) as pP, \
                     tc.tile_pool(name="pFq", bufs=4) as pFq, \
                     tc.tile_pool(name="pPq", bufs=4) as pPq, \
                     tc.tile_pool(name="pOsb", bufs=2) as pOsb:
                    # consumption sequence: TAP_ORDER per tile, pool last.
                    # production sequence: pool-taps hoisted one tile early
                    # (their inputs are phase-A products, and the Pool's
                    # slow multiply needs the head start).
                    # tile 3's drain: consume the slow-produced ACT-flt
                    # taps mid-tile and the fast DVE-ts taps last, so the
                    # pipeline wind-down stays PE-bound.
                    TAP_ORDER_L = (2, 8, 4, 6, 7, 0, 3, 5, 1)
                    def order_of(t):
                        return TAP_ORDER_L
                    cons_seq = [(t, k) for t in range(NST)
                                for k in order_of(t)]
                    prod_seq = []
                    for t in range(NST):
                        reg = [k for k in order_of(t)
                               if k not in POOL_TT_TAPS]
                        # first three regular taps lead (shortest chain
                        # to the first wfbs), then the pool taps
                        prod_seq += [(t, k) for k in reg[:3]]
                        if t == 0:
                            prod_seq += [(0, k) for k in POOL_TT_TAPS]
                        prod_seq += [(t, k) for k in reg[3:]]
                        if t + 1 < NST:
                            prod_seq += [(t + 1, k)
                                         for k in POOL_TT_TAPS]
                    p_tiles = {}
                    out_tiles = {}

                    def produce(t, k):
                        dh, dw = divmod(k, 3)
                        xwin = xpad[:, 16 * t + dh:16 * t + dh + ROWS,
                                    dw:dw + W]
                        pool_tap = k in POOL_TT_TAPS
                        pPx = pPq if pool_tap else pP
                        pFx = pFq if pool_tap else pF
                        p_sb = pPx.tile([C, ROWS, W], BF16, tag="p")
                        if USE_DMA_BCAST and k in DMA_TAPS:
                            # broadcast arrived by DMA
                            di = DMA_TAPS.index(k)
                            sb = sbc[:, t, di].rearrange(
                                "c (r w) -> c r w", w=W)
                            flt = pFx.tile([C, ROWS, W], BF16, tag="flt")
                            if k == ACT_SBUF_TAP or (t == 0 and k == 8):
                                # ACT takes this add for DVE/ACT balance
                                _lbl(nc.scalar.activation(
                                    out=flt, in_=sb, func=ACT_IDENT,
                                    bias=cfsb[:, k:k + 1]),
                                     f"fltA({t},{k})")
                            else:
                                _lbl(nc.vector.tensor_scalar(
                                    out=flt, in0=sb,
                                    scalar1=cfsb[:, k:k + 1],
                                    scalar2=1.0, op0=ADD, op1=MULT),
                                     f"ts({t},{k})")
                            eng = nc.gpsimd if pool_tap else nc.vector
                            _lbl(eng.tensor_tensor(
                                out=p_sb, in0=flt, in1=xwin, op=MULT),
                                 f"tt({t},{k})")
                        else:
                            emit_bc(t, k)
                            bc_ps = bc_tiles.pop((t, k))
                            if k in DMA_TAPS:
                                _lbl(nc.vector.scalar_tensor_tensor(
                                    out=p_sb, in0=bc_ps,
                                    scalar=cfsb[:, k:k + 1], in1=xwin,
                                    op0=ADD, op1=MULT), f"stt({t},{k})")
                            else:
                                # ACT adds cf (bias) -> bf16 SBUF, DVE
                                # multiplies at 2x
                                flt = pFx.tile([C, ROWS, W], BF16,
                                               tag="flt")
                                _lbl(nc.scalar.activation(
                                    out=flt, in_=bc_ps, func=ACT_IDENT,
                                    bias=cfsb[:, k:k + 1]),
                                     f"fltP({t},{k})")
                                _lbl(nc.vector.tensor_tensor(
                                    out=p_sb, in0=flt, in1=xwin,
                                    op=MULT), f"tt({t},{k})")
                        p_tiles[(t, k)] = p_sb

                    def consume(t, k, ki):
                        if ki == 0:
                            ohs = []
                            for h in range(2):
                                out_ps = psO.tile([C, 8, W], F32,
                                                  tag=f"out{h}")
                                ohs.append(out_ps)
                                if h == 0:
                                    _absorb(
                                        nc,
                                        xs[0:1, t * STN:t * STN + 1],
                                        out_ps[0:1, 0, 0:1])
                                c0 = t * STN + h * 512
                                nc.tensor.matmul(
                                    out_ps, t_wfa, xs[:, c0:c0 + 512],
                                    start=True, stop=False)
                            out_tiles[t] = ohs
                        ohs = out_tiles[t]
                        p_sb = p_tiles.pop((t, k))
                        for h in range(2):
                            _lbl(nc.tensor.matmul(
                                ohs[h], t_wfb,
                                p_sb[:, 8 * h:8 * h + 8, :],
                                start=False, stop=(ki == NT - 1)),
                                 f"wfb({t},{k},{h})")
                        if ki == NT - 1:
                            last = t == NST - 1
                            for h in range(2):
                                o_sb = pOsb.tile([C, 8, W], BF16,
                                                 tag=f"osb{h}")
                                if last and h == 1:
                                    _lbl(nc.vector.tensor_copy(
                                        out=o_sb, in_=ohs[h]),
                                         f"osb({t},{h})")
                                else:
                                    _lbl(nc.scalar.copy(out=o_sb,
                                                        in_=ohs[h]),
                                         f"osb({t},{h})")
                                # last tile: DMAs on both HWDGE queues so
                                # their issue overheads overlap
                                eng = nc.scalar if (last and h == 1) \
                                    else nc.sync
                                _lbl(eng.dma_start(
                                    out=ob[:, 16 * t + 8 * h:
                                           16 * t + 8 * h + 8, :],
                                    in_=o_sb.rearrange(
                                        "c r w -> c (r w)"),
                                ), f"outdma({t},{h})")

                    for j in range(len(prod_seq) + LEAD):
                        if j < len(prod_seq):
                            produce(*prod_seq[j])
                        if j >= LEAD:
                            t, k = cons_seq[j - LEAD]
                            consume(t, k, (j - LEAD) % NT)
    _split_multiwaits(nc)
    return nc


def _bf16(a):
    return np.asarray(a, dtype=np.float32).astype(ml_dtypes.bfloat16)


def _prep_weights(static_w, w1, w2, w3, ws, wf):
    """Repack the tiny weights into the SBUF layouts the kernel expects.
    Returns the [C, WA_COLS + WC_COLS + 2*CTXF32] bf16 weight tail."""
    f = np.float32
    sw = np.ascontiguousarray(static_w.reshape(C, NT), dtype=f)

    dsw = np.zeros((C, NT * C), dtype=f)
    for k in range(NT):
        dsw[np.arange(C), k * C + np.arange(C)] = sw[:, k]

    wsa = np.zeros((C, MREP), dtype=f)
    wsb = np.zeros((C, MREP), dtype=f)
    for g in range(2):
        for k in range(NT):
            wsa[:, 32 * g + k] = ws[k, :C]
            wsb[:, 32 * g + k] = ws[k, C:]

    wfa = np.ascontiguousarray(wf[:, :C].T, dtype=f)
    wfb = np.ascontiguousarray(wf[:, C:].T, dtype=f)

    bct = np.zeros((C, NT * C), dtype=f)
    for g in range(2):
        for k in range(NT):
            bct[32 * g + k, k * C:(k + 1) * C] = 1.0

    tail16 = _bf16(np.concatenate([dsw, wsa, wsb, wfa, wfb, bct], axis=1))

    # fp32 context weights (1/HW folded into w1), bitcast into bf16 pairs
    w1s = np.asarray(w1, dtype=f) * (1.0 / HW)
    ctx = np.zeros((C, CTXF32), dtype=f)
    ctx[:, 0:64] = w1s[:, :C].T
    ctx[:, 64:128] = w1s[:, C:].T
    ctx[0:64, 128:192] = np.asarray(w2, dtype=f).T
    w3t = np.ascontiguousarray(
        np.asarray(w3, dtype=f).reshape(C, NT, 64).transpose(2, 1, 0)
    ).reshape(64, NT * C)
    ctx[0:64, 192:192 + NT * C] = w3t
    ctx16 = np.ascontiguousarray(ctx).view(ml_dtypes.bfloat16)

    return np.concatenate([tail16, ctx16], axis=1)


def make_in_maps(X2, Y2, static_w, w1, w2, w3, ws, wf):
    wpack = _prep_weights(
        np.asarray(static_w), np.asarray(w1), np.asarray(w2),
        np.asarray(w3), np.asarray(ws), np.asarray(wf),
    )
    X2 = np.asarray(X2)
    Y2 = np.asarray(Y2)
    xpad_all = np.zeros((B, C, PH, PW), dtype=np.float32)
    xpad_all[:, :, 1:H + 1, 1:W + 1] = X2
    xpad_all = _bf16(xpad_all).reshape(B, C, PH * PW)
    y2_all = _bf16(Y2.reshape(B, C, HW))
    in_maps = []
    for b in range(B):
        m = {"pk": np.ascontiguousarray(np.concatenate(
            [xpad_all[b], y2_all[b], wpack], axis=1))}
        in_maps.append(m)
    return in_maps


def get_nc():
    if "nc" not in _CACHE:
        _CACHE["nc"] = _build_bass()
    return _CACHE["nc"]


def kernel(X2, Y2, static_w, w1, w2, w3, ws, wf):
    nc = get_nc()
    in_maps = make_in_maps(
        np.asarray(X2), np.asarray(Y2), static_w, w1, w2, w3, ws, wf)
    res = run_bass_kernel_spmd(nc, in_maps, core_ids=list(range(B)))
    out = np.stack([np.asarray(r["ob"]).astype(np.float32)
                    for r in res.results])
    return out
